# revision 6
# baseline (speedup 1.0000x reference)
"""Trainium2 Bass kernel for DWConvBlock3D:
depthwise 3x3x3 conv (pad 1) + InstanceNorm3d + ReLU on x:(2,64,64,128,128) f32.

Strategy (8 NeuronCores, channel sharding => zero communication):
  - Each core owns 8 channels x 2 batches = 16 (b,c) "pairs".
  - Layout per pair: H=128 on SBUF partitions, (D,W) on the free dim, with
    host-side zero padding in both D (66) and W (130) so every matmul is
    uniform (no edge clipping).
  - 14 pairs run on TensorE as banded matmuls: a 128x128 banded matrix
    (3 diagonals = the kh taps) multiplies a (d,w)-shifted view of the x
    tile; shifts cover (kd,kw) -> 9 matmuls per 512-col chunk.
  - 2 pairs run on the otherwise-idle DVE: the host ships 3 h-shifted
    copies (split in d-halves to bound SBUF) so all 27 taps are
    free-dim-shifted ops.  Aligned taps (kw 0/2) use tensor_scalar (4x)
    + tensor_tensor add (2x) = 6.5us/tap-volume; kw=1 taps (odd offset,
    no 2x uop for STT) use scalar_tensor_tensor at 1x.  Ops are
    interleaved a few per PE-pair section so the DVE queue never starves.
  - PSUM: mm pool 2 bufs x 3 banks (groups of 3 chunks; eviction =
    ScalarE activation-copy fp32->fp16 with accum_out -> sum(y)); plus a
    1-bank "red" pool for the stats reduction.
  - sum(y^2): ScalarE Square-activation per group (accum_out), scrap
    output discarded -> keeps DVE free for conv taps.
  - cross-partition reduction of (sum, sumsq): a tiny fp32 matmul with
    an all-ones stationary (built on-chip by memset) sums over partitions
    AND broadcasts to all 128 output partitions in ~0.5us, replacing the
    5.7us GpSimd partition_all_reduce.
  - final normalize+ReLU: ScalarE activation with per-partition
    scale/bias; output DMA'd as fp16.

Measured notes (HW probes): matmul streams 1 col/cycle @2.4GHz for all
dtypes; fp8 DoubleRow/DoublePixel give no column-rate win (DoublePixel is
silently ignored); DVE STT has only a 1x uop (8.7us per full volume);
TT=2x, TS=4x; ScalarE is strictly 1x (no Accel modes).
"""

import sys

if "/opt/trn_rl_repo" not in sys.path:
    sys.path.insert(0, "/opt/trn_rl_repo")

import numpy as np

B, C, D, H, W = 2, 64, 64, 128, 128
N_CORES = 8
CH_PER_CORE = C // N_CORES  # 8
N_PAIRS = B * CH_PER_CORE  # 16
DP = D + 2  # host-padded D
WP = W + 2  # host-padded W
FREE = D * W  # 8192 output cols per partition per pair
NV = D * H * W  # normalization element count per (b,c)
EPS = 1e-5
CD = 4  # d-slices per chunk (4*128 = 512 fp32 = 1 PSUM bank)
GROUP_SIZES = [4, 4, 4, 4]  # chunks per PSUM group (4 banks each)
N_CHUNKS = D // CD  # 16
N_GROUPS = len(GROUP_SIZES)  # 6
TAPS = [(kd, kw) for kd in range(3) for kw in range(3)]
N_DVE_PAIRS = 2
N_PE_PAIRS = N_PAIRS - N_DVE_PAIRS  # 14
DH = D // 2  # d-half for DVE pairs
DHP = DH + 2  # padded rows per half


def build_program():
    import concourse.bacc as bacc
    import concourse.mybir as mybir
    from concourse.tile import TileContext

    from concourse import bass_isa

    f32 = mybir.dt.float32
    f16 = mybir.dt.float16
    AF = mybir.ActivationFunctionType
    OP = mybir.AluOpType
    nc = bacc.Bacc("TRN2", target_bir_lowering=False, debug=False, num_devices=N_CORES)

    xs = nc.dram_tensor("xs", [N_PE_PAIRS, H, DP, WP], f16, kind="ExternalInput")
    bands = nc.dram_tensor("bands", [H, CH_PER_CORE, 9, H], f16, kind="ExternalInput")
    gb = nc.dram_tensor("gb", [128, 2 * N_PAIRS], f32, kind="ExternalInput")
    # DVE pairs: 3 h-shifted copies, split in two d-halves (34 padded rows)
    xd = nc.dram_tensor("xd", [N_DVE_PAIRS, 2, H, 3, DHP, WP], f16,
                        kind="ExternalInput")
    wd = nc.dram_tensor("wd", [128, N_DVE_PAIRS * 27], f32, kind="ExternalInput")
    out = nc.dram_tensor("out", [N_PAIRS, H, FREE], f16, kind="ExternalOutput")

    with TileContext(nc) as tc:
        with (
            tc.tile_pool(name="singles", bufs=1) as singles,
            tc.tile_pool(name="xp", bufs=2) as xpool,
            tc.tile_pool(name="yp", bufs=3) as ypool,
            tc.tile_pool(name="xd", bufs=2) as xdpool,
            tc.tile_pool(name="yd", bufs=2) as ydpool,
            tc.tile_pool(name="st", bufs=4) as stats,
            tc.tile_pool(name="psmm", bufs=2, space="PSUM") as psum_mm,
        ):
            band_sb = singles.tile([H, CH_PER_CORE, 9, H], f16)
            gb_sb = singles.tile([128, 2 * N_PAIRS], f32)
            wd_sb = singles.tile([128, N_DVE_PAIRS * 27], f32)
            scrap = singles.tile([128, 4 * CD * W], f16)  # Square-act output
            sq4 = singles.tile([128, DH * W], f16)  # Square output, DVE halves
            tmp = singles.tile([H, DH * W], f16)  # DVE TS scratch (half vol)

            # ---------- DVE-pair machinery ----------
            dve_ops = []  # list of closures, emitted a few per PE section

            def plan_dve_pair(dp):
                """Queue all DVE ops for DVE-pair index dp (pair N_PE_PAIRS+dp)."""
                p = N_PE_PAIRS + dp
                yd_t = ydpool.tile([H, FREE], f16, tag="yd", name=f"yd{dp}")
                sd = stats.tile([128, 2, 2], f32, tag=f"sd{dp}")
                xh_t = [None, None]

                def load_half(h):
                    xh_t[h] = xdpool.tile([H, 3, DHP, WP], f16, tag="xdh",
                                          name=f"xd{dp}_{h}")
                    nc.sync.dma_start(out=xh_t[h][:], in_=xd[dp, h])

                def do_tap(h, t, first, last):
                    kd, kh, kw = t // 9, (t // 3) % 3, t % 3
                    view = xh_t[h][:, kh, kd : kd + DH, kw : kw + W]
                    ycur = yd_t[:, h * DH * W : (h + 1) * DH * W]
                    wcol = wd_sb[:, dp * 27 + t : dp * 27 + t + 1]
                    if first:
                        nc.vector.tensor_scalar_mul(ycur, view, wcol)
                    elif kw == 1:
                        nc.vector.scalar_tensor_tensor(
                            out=ycur, in0=view, scalar=wcol, in1=ycur,
                            op0=OP.mult, op1=OP.add,
                            accum_out=sd[:, 0, h : h + 1] if last else None,
                        )
                    else:
                        nc.vector.tensor_scalar_mul(tmp[:], view, wcol)
                        nc.vector.tensor_tensor(
                            out=ycur, in0=ycur, in1=tmp[:], op=OP.add
                        )

                def do_sumsq(h):
                    ycur = yd_t[:, h * DH * W : (h + 1) * DH * W]
                    nc.scalar.activation(
                        out=sq4[:], in_=ycur, func=AF.Square,
                        accum_out=sd[:, 1, h : h + 1],
                    )

                # aligned taps (kw 0/2) first, kw=1 (STT, in-place) last so the
                # final op carries accum_out for sum(y)
                order = [t for t in range(27) if t % 3 != 1] + \
                        [t for t in range(27) if t % 3 == 1]
                for h in range(2):
                    dve_ops.append(lambda h=h: load_half(h))
                    for i, t in enumerate(order):
                        dve_ops.append(
                            lambda h=h, t=t, fi=(i == 0), la=(i == 26):
                            do_tap(h, t, fi, la)
                        )
                    dve_ops.append(lambda h=h: do_sumsq(h))

                def finish():
                    st2 = stats.tile([128, 2], f32, tag="st2d")
                    nc.vector.tensor_reduce(
                        out=st2[:], in_=sd[:], axis=mybir.AxisListType.X,
                        op=OP.add,
                    )
                    red = stats_phase1(st2)
                    sb2 = stats_phase2(p, red)
                    for h2 in range(2):
                        norm_slice(p, yd_t[:], sb2, h2, 2)

                dve_ops.append(finish)

            def emit_dve(n):
                for _ in range(n):
                    if dve_ops:
                        dve_ops.pop(0)()

            # ---------- shared stats tail ----------
            def stats_phase1(st2):
                """cross-partition reduce of (sum, sumsq) on GpSimd (async)."""
                red = stats.tile([128, 2], f32, tag="red")
                nc.gpsimd.partition_all_reduce(
                    red[:], st2[:], 128, bass_isa.ReduceOp.add
                )
                return red

            def stats_phase2(p, red):
                """red [128,2] broadcast (N*mean, N*ex2) -> sc/bi tiles."""
                sm = stats.tile([128, 10], f32, tag="sm")
                mean, ex2 = sm[:, 0:1], sm[:, 1:2]
                msq, vpe = sm[:, 2:3], sm[:, 3:4]
                std, r0 = sm[:, 4:5], sm[:, 5:6]
                t1, t2 = sm[:, 6:7], sm[:, 7:8]
                t4, rr = sm[:, 8:9], sm[:, 9:10]
                nc.vector.tensor_scalar_mul(mean, red[:, 0:1], 1.0 / NV)
                nc.vector.tensor_scalar_mul(ex2, red[:, 1:2], 1.0 / NV)
                nc.vector.tensor_mul(msq, mean, mean)
                nc.vector.tensor_sub(vpe, ex2, msq)
                nc.vector.tensor_scalar_add(vpe, vpe, EPS)
                nc.scalar.activation(std, vpe, mybir.ActivationFunctionType.Sqrt)
                nc.vector.reciprocal(r0, std)
                # one Newton step: r = r0*(1.5 - 0.5*vpe*r0^2)
                nc.vector.tensor_mul(t1, r0, r0)
                nc.vector.tensor_mul(t2, t1, vpe)
                nc.vector.tensor_scalar(
                    t4, t2, -0.5, 1.5, op0=OP.mult, op1=OP.add,
                )
                nc.vector.tensor_mul(rr, r0, t4)

                sb2 = stats.tile([128, 2], f32, tag="sb2")
                sc, bi = sb2[:, 0:1], sb2[:, 1:2]
                # scale = gamma * rstd ; bias = beta - mean*scale
                nc.vector.tensor_mul(sc, rr, gb_sb[:, p : p + 1])
                nc.vector.tensor_mul(t1, mean, sc)
                nc.vector.tensor_sub(
                    bi, gb_sb[:, N_PAIRS + p : N_PAIRS + p + 1], t1
                )
                return sb2

            def norm_slice(p, y_flat, sb2, h2, nq):
                qf = FREE // nq
                sc, bi = sb2[:, 0:1], sb2[:, 1:2]
                ysl = y_flat[:, h2 * qf : (h2 + 1) * qf]
                nc.scalar.activation(
                    out=ysl, in_=ysl, func=AF.Relu, scale=sc, bias=bi,
                )
                nc.gpsimd.dma_start(
                    out=out[p][:, h2 * qf : (h2 + 1) * qf], in_=ysl
                )

            # ---------- main PE-pair loop (software-pipelined stats) ----------
            gb_loaded = False
            pend = [None]  # (p, y, sb2) awaiting normalize
            for p in range(N_PE_PAIRS):
                ci = p % CH_PER_CORE

                if p < CH_PER_CORE:
                    nc.sync.dma_start(out=band_sb[:, ci], in_=bands[:, ci])
                xt = xpool.tile([H, DP, WP], f16, tag="xt")
                nc.sync.dma_start(out=xt[:], in_=xs[p])
                if not gb_loaded:
                    nc.sync.dma_start(out=gb_sb[:], in_=gb[:])
                    nc.sync.dma_start(out=wd_sb[:], in_=wd[:])
                    gb_loaded = True
                if p == 0:
                    plan_dve_pair(0)
                if p == 5:
                    plan_dve_pair(1)

                y = ypool.tile([H, FREE], f16, tag="y")
                ss = stats.tile([128, 2, N_GROUPS], f32, tag="ss")
                st2 = stats.tile([128, 2], f32, tag="st2")

                # ---- depthwise conv: banded matmuls, PSUM-accumulated
                c0 = 0
                for g, gsz_c in enumerate(GROUP_SIZES):
                    gsz = gsz_c * CD * W
                    ps = psum_mm.tile([128, 4 * CD * W], f32, tag="mm",
                                      name=f"mm_{p}_{g}")
                    for j in range(9):
                        kd, kw = TAPS[j]
                        for c in range(gsz_c):
                            d0 = (c0 + c) * CD
                            nc.tensor.matmul(
                                ps[:, c * CD * W : (c + 1) * CD * W],
                                band_sb[:, ci, j],
                                xt[:, d0 + kd : d0 + kd + CD, kw : kw + W],
                                start=(j == 0), stop=(j == 8),
                                skip_group_check=True,
                            )
                    # ---- evict group: PSUM -> y (fp16), sum(y) via accum_out
                    ysl = y[:, c0 * CD * W : c0 * CD * W + gsz]
                    nc.scalar.activation(
                        out=ysl, in_=ps[:, 0:gsz], func=AF.Copy,
                        accum_out=ss[:, 0, g : g + 1],
                    )
                    # ---- sum(y^2) for the group: ScalarE Square, discard out
                    nc.scalar.activation(
                        out=scrap[:, 0:gsz], in_=ysl, func=AF.Square,
                        accum_out=ss[:, 1, g : g + 1],
                    )
                    c0 += gsz_c
                    # previous pair's normalize, spread across this section
                    if pend[0] is not None and g in (1, 2):
                        pp_, py_, psb_ = pend[0]
                        norm_slice(pp_, py_, psb_, g - 1, 2)
                        if g == 2:
                            pend[0] = None

                # feed the DVE queue BEFORE the reduce: ops behind a waiting
                # queue head cannot run, so taps must be ahead of it
                emit_dve(2 if p == 0 else 12)
                nc.vector.tensor_reduce(
                    out=st2[:], in_=ss[:], axis=mybir.AxisListType.X, op=OP.add,
                )
                red = stats_phase1(st2)
                emit_dve(2)
                sb2 = stats_phase2(p, red)
                if p < N_PE_PAIRS - 1:
                    pend[0] = (p, y, sb2)
                else:
                    # last pair: normalize immediately, finer slices
                    for h2 in range(4):
                        norm_slice(p, y[:], sb2, h2, 4)
            emit_dve(len(dve_ops))

    nc.compile()
    return nc


_NC_CACHE = None


def _get_program():
    global _NC_CACHE
    if _NC_CACHE is None:
        _NC_CACHE = build_program()
    return _NC_CACHE


def _band_mats(w, c):
    """f32 band matrices [9, H, H] for channel c, tap order TAPS."""
    eye0 = np.eye(H, dtype=np.float32)
    eyep = np.eye(H, k=1, dtype=np.float32)  # B[h-1, h]: kh=0 tap
    eyem = np.eye(H, k=-1, dtype=np.float32)  # B[h+1, h]: kh=2 tap
    mats = np.empty((9, H, H), np.float32)
    for t, (kd, kw) in enumerate(TAPS):
        wk = w[c, 0, kd, :, kw]
        mats[t] = wk[0] * eyep + wk[1] * eye0 + wk[2] * eyem
    return mats


def make_core_inputs(x, w, gamma, beta, core):
    cs = slice(CH_PER_CORE * core, CH_PER_CORE * (core + 1))
    # (b, ci, d, h, w) -> (b, ci, h, d, w) -> (pair, h, d, w), pair = b*8+ci
    xt = (
        np.ascontiguousarray(x[:, cs].transpose(0, 1, 3, 2, 4))
        .reshape(N_PAIRS, H, D, W)
    )
    xc = np.zeros((N_PE_PAIRS, H, DP, WP), np.float16)
    xc[:, :, 1 : D + 1, 1 : W + 1] = xt[:N_PE_PAIRS].astype(np.float16)
    bpk = np.zeros((H, CH_PER_CORE, 9, H), np.float32)
    for ci in range(CH_PER_CORE):
        bpk[:, ci] = _band_mats(w, CH_PER_CORE * core + ci).transpose(1, 0, 2)
    band_arr = bpk.astype(np.float16)
    gbv = np.broadcast_to(
        np.concatenate([np.tile(gamma[cs], B), np.tile(beta[cs], B)])
        .astype(np.float32)
        .reshape(1, 2 * N_PAIRS),
        (128, 2 * N_PAIRS),
    ).copy()

    # DVE pairs: 3 h-shifted padded copies, two d-halves each
    xdv = np.zeros((N_DVE_PAIRS, 2, H, 3, DHP, WP), np.float16)
    wcols = np.zeros((128, N_DVE_PAIRS * 27), np.float32)
    for dp in range(N_DVE_PAIRS):
        pair = N_PE_PAIRS + dp
        b_i, ci = pair // CH_PER_CORE, pair % CH_PER_CORE
        ch = CH_PER_CORE * core + ci
        xp = np.zeros((H + 2, DP, WP), np.float16)
        xp[1 : H + 1, 1 : D + 1, 1 : W + 1] = (
            x[b_i, ch].transpose(1, 0, 2).astype(np.float16)
        )
        for k in range(3):
            shifted = xp[k : k + H]  # h+ (k-1)
            for h in range(2):
                xdv[dp, h, :, k] = shifted[:, h * DH : h * DH + DHP]
        wcols[:, dp * 27 : (dp + 1) * 27] = np.broadcast_to(
            w[ch, 0].reshape(1, 27).astype(np.float32), (128, 27)
        )
    return {"xs": xc, "bands": band_arr, "gb": gbv, "xd": xdv, "wd": wcols}


def kernel(x, w, gamma, beta):
    from concourse.bass_utils import run_bass_kernel_spmd

    x = np.asarray(x, dtype=np.float32)
    w = np.asarray(w, dtype=np.float32)
    gamma = np.asarray(gamma, dtype=np.float32)
    beta = np.asarray(beta, dtype=np.float32)

    nc = _get_program()
    in_maps = [make_core_inputs(x, w, gamma, beta, k) for k in range(N_CORES)]
    res = run_bass_kernel_spmd(nc, in_maps, core_ids=list(range(N_CORES)))

    out = np.empty((B, C, D, H, W), np.float32)
    for k in range(N_CORES):
        cs = slice(CH_PER_CORE * k, CH_PER_CORE * (k + 1))
        yc = (
            res.results[k]["out"]
            .astype(np.float32)
            .reshape(B, CH_PER_CORE, H, D, W)
        )
        out[:, cs] = yc.transpose(0, 1, 3, 2, 4)
    return out


# revision 8
# speedup vs baseline: 1.0404x; 1.0404x over previous
"""Trainium2 Bass kernel for DWConvBlock3D:
depthwise 3x3x3 conv (pad 1) + InstanceNorm3d + ReLU on x:(2,64,64,128,128) f32.

Strategy (8 NeuronCores, channel sharding => zero communication):
  - Each core owns 8 channels x 2 batches = 16 (b,c) "pairs".
  - Layout per pair: H=128 on SBUF partitions, (D,W) on the free dim, with
    host-side zero padding in both D (66) and W (130) so every matmul is
    uniform (no edge clipping).
  - 14 pairs run on TensorE as banded matmuls: a 128x128 banded matrix
    (3 diagonals = the kh taps) multiplies a (d,w)-shifted view of the x
    tile; shifts cover (kd,kw) -> 9 matmuls per 512-col chunk.
  - 2 pairs run on the otherwise-idle DVE: the host ships 3 h-shifted
    copies (split in d-halves to bound SBUF) so all 27 taps are
    free-dim-shifted ops.  Aligned taps (kw 0/2) use tensor_scalar (4x)
    + tensor_tensor add (2x) = 6.5us/tap-volume; kw=1 taps (odd offset,
    no 2x uop for STT) use scalar_tensor_tensor at 1x.  Ops are
    interleaved a few per PE-pair section so the DVE queue never starves.
  - PSUM: mm pool 2 bufs x 4 banks (groups of 4 chunks; eviction =
    ScalarE activation-copy fp32->fp16 with accum_out -> sum(y)).
  - sum(y^2): ScalarE Square-activation per group (accum_out), scrap
    output discarded -> keeps DVE free for conv taps.
  - cross-partition reduction of (sum, sumsq): GpSimd partition_all_reduce
    issued as "phase1" right after the per-pair reduce, with conv taps
    emitted between it and the dependent "phase2" scale/bias math so the
    DVE never idles on the GpSimd latency; the normalize of pair p is
    deferred into pair p+1's section (emitted after its early evictions)
    so a waiting normalize never head-blocks the ScalarE FIFO.
  - final normalize+ReLU: ScalarE activation with per-partition
    scale/bias; output DMA'd as fp16.

Measured notes (HW probes): matmul streams 1 col/cycle @2.4GHz for all
dtypes; fp8 DoubleRow/DoublePixel give no column-rate win (DoublePixel is
silently ignored); DVE STT has only a 1x uop (8.7us per full volume);
TT=2x, TS=4x; ScalarE is strictly 1x (no Accel modes).
"""

import sys

if "/opt/trn_rl_repo" not in sys.path:
    sys.path.insert(0, "/opt/trn_rl_repo")

import numpy as np

B, C, D, H, W = 2, 64, 64, 128, 128
N_CORES = 8
CH_PER_CORE = C // N_CORES  # 8
N_PAIRS = B * CH_PER_CORE  # 16
DP = D + 2  # host-padded D
WP = W + 2  # host-padded W
FREE = D * W  # 8192 output cols per partition per pair
NV = D * H * W  # normalization element count per (b,c)
EPS = 1e-5
CD = 4  # d-slices per chunk (4*128 = 512 fp32 = 1 PSUM bank)
GROUP_SIZES = [4, 4, 4, 4]  # chunks per PSUM group (4 banks each)
N_CHUNKS = D // CD  # 16
N_GROUPS = len(GROUP_SIZES)  # 6
TAPS = [(kd, kw) for kd in range(3) for kw in range(3)]
N_DVE_PAIRS = 2
N_PE_PAIRS = N_PAIRS - N_DVE_PAIRS  # 14
DH = D // 2  # d-half for DVE pairs
DHP = DH + 2  # padded rows per half


def build_program():
    import concourse.bacc as bacc
    import concourse.mybir as mybir
    from concourse.tile import TileContext

    from concourse import bass_isa

    f32 = mybir.dt.float32
    f16 = mybir.dt.float16
    AF = mybir.ActivationFunctionType
    OP = mybir.AluOpType
    nc = bacc.Bacc("TRN2", target_bir_lowering=False, debug=False, num_devices=N_CORES)

    xs = nc.dram_tensor("xs", [N_PE_PAIRS, H, DP, WP], f16, kind="ExternalInput")
    bands = nc.dram_tensor("bands", [H, CH_PER_CORE, 9, H], f16, kind="ExternalInput")
    gb = nc.dram_tensor("gb", [128, 2 * N_PAIRS], f32, kind="ExternalInput")
    # DVE pairs: 3 h-shifted copies, split in two d-halves (34 padded rows)
    xd = nc.dram_tensor("xd", [N_DVE_PAIRS, 2, H, 3, DHP, WP], f16,
                        kind="ExternalInput")
    wd = nc.dram_tensor("wd", [128, N_DVE_PAIRS * 27], f32, kind="ExternalInput")
    out = nc.dram_tensor("out", [N_PAIRS, H, FREE], f16, kind="ExternalOutput")

    with TileContext(nc) as tc:
        with (
            tc.tile_pool(name="singles", bufs=1) as singles,
            tc.tile_pool(name="xp", bufs=2) as xpool,
            tc.tile_pool(name="yp", bufs=3) as ypool,
            tc.tile_pool(name="xd", bufs=2) as xdpool,
            tc.tile_pool(name="yd", bufs=2) as ydpool,
            tc.tile_pool(name="st", bufs=4) as stats,
            tc.tile_pool(name="psmm", bufs=2, space="PSUM") as psum_mm,
        ):
            band_sb = singles.tile([H, CH_PER_CORE, 9, H], f16)
            gb_sb = singles.tile([128, 2 * N_PAIRS], f32)
            wd_sb = singles.tile([128, N_DVE_PAIRS * 27], f32)
            scrap = singles.tile([128, 4 * CD * W], f16)  # Square-act output
            sq4 = singles.tile([128, DH * W], f16)  # Square output, DVE halves
            tmp = singles.tile([H, DH * W], f16)  # DVE TS scratch (half vol)

            # ---------- DVE-pair machinery ----------
            dve_ops = []  # list of closures, emitted a few per PE section

            def plan_dve_pair(dp):
                """Queue all DVE ops for DVE-pair index dp (pair N_PE_PAIRS+dp)."""
                p = N_PE_PAIRS + dp
                yd_t = ydpool.tile([H, FREE], f16, tag="yd", name=f"yd{dp}")
                sd = stats.tile([128, 2, 2], f32, tag=f"sd{dp}")
                xh_t = [None, None]

                def load_half(h):
                    xh_t[h] = xdpool.tile([H, 3, DHP, WP], f16, tag="xdh",
                                          name=f"xd{dp}_{h}")
                    nc.sync.dma_start(out=xh_t[h][:], in_=xd[dp, h])

                def do_tap(h, t, first, last):
                    kd, kh, kw = t // 9, (t // 3) % 3, t % 3
                    view = xh_t[h][:, kh, kd : kd + DH, kw : kw + W]
                    ycur = yd_t[:, h * DH * W : (h + 1) * DH * W]
                    wcol = wd_sb[:, dp * 27 + t : dp * 27 + t + 1]
                    if first:
                        nc.vector.tensor_scalar_mul(ycur, view, wcol)
                    elif kw == 1:
                        nc.vector.scalar_tensor_tensor(
                            out=ycur, in0=view, scalar=wcol, in1=ycur,
                            op0=OP.mult, op1=OP.add,
                            accum_out=sd[:, 0, h : h + 1] if last else None,
                        )
                    else:
                        nc.vector.tensor_scalar_mul(tmp[:], view, wcol)
                        nc.vector.tensor_tensor(
                            out=ycur, in0=ycur, in1=tmp[:], op=OP.add
                        )

                def do_sumsq(h):
                    ycur = yd_t[:, h * DH * W : (h + 1) * DH * W]
                    nc.scalar.activation(
                        out=sq4[:], in_=ycur, func=AF.Square,
                        accum_out=sd[:, 1, h : h + 1],
                    )

                # aligned taps (kw 0/2) first, kw=1 (STT, in-place) last so the
                # final op carries accum_out for sum(y)
                order = [t for t in range(27) if t % 3 != 1] + \
                        [t for t in range(27) if t % 3 == 1]
                for h in range(2):
                    dve_ops.append(lambda h=h: load_half(h))
                    for i, t in enumerate(order):
                        dve_ops.append(
                            lambda h=h, t=t, fi=(i == 0), la=(i == 26):
                            do_tap(h, t, fi, la)
                        )
                    dve_ops.append(lambda h=h: do_sumsq(h))

                def finish():
                    st2 = stats.tile([128, 2], f32, tag="st2d")
                    nc.vector.tensor_reduce(
                        out=st2[:], in_=sd[:], axis=mybir.AxisListType.X,
                        op=OP.add,
                    )
                    red = stats_phase1(st2)
                    sb2 = stats_phase2(p, red)
                    for h2 in range(2):
                        norm_slice(p, yd_t[:], sb2, h2, 2)

                dve_ops.append(finish)

            def emit_dve(n):
                for _ in range(n):
                    if dve_ops:
                        dve_ops.pop(0)()

            # ---------- shared stats tail ----------
            def stats_phase1(st2):
                """cross-partition reduce of (sum, sumsq) on GpSimd (async)."""
                red = stats.tile([128, 2], f32, tag="red")
                nc.gpsimd.partition_all_reduce(
                    red[:], st2[:], 128, bass_isa.ReduceOp.add
                )
                return red

            def stats_phase2(p, red):
                """red [128,2] broadcast (N*mean, N*ex2) -> sc/bi tiles."""
                sm = stats.tile([128, 10], f32, tag="sm")
                mean, ex2 = sm[:, 0:1], sm[:, 1:2]
                msq, vpe = sm[:, 2:3], sm[:, 3:4]
                std, r0 = sm[:, 4:5], sm[:, 5:6]
                t1, t2 = sm[:, 6:7], sm[:, 7:8]
                t4, rr = sm[:, 8:9], sm[:, 9:10]
                nc.vector.tensor_scalar_mul(mean, red[:, 0:1], 1.0 / NV)
                nc.vector.tensor_scalar_mul(ex2, red[:, 1:2], 1.0 / NV)
                nc.vector.tensor_mul(msq, mean, mean)
                nc.vector.tensor_sub(vpe, ex2, msq)
                nc.vector.tensor_scalar_add(vpe, vpe, EPS)
                nc.scalar.activation(std, vpe, mybir.ActivationFunctionType.Sqrt)
                nc.vector.reciprocal(r0, std)
                # one Newton step: r = r0*(1.5 - 0.5*vpe*r0^2)
                nc.vector.tensor_mul(t1, r0, r0)
                nc.vector.tensor_mul(t2, t1, vpe)
                nc.vector.tensor_scalar(
                    t4, t2, -0.5, 1.5, op0=OP.mult, op1=OP.add,
                )
                nc.vector.tensor_mul(rr, r0, t4)

                sb2 = stats.tile([128, 2], f32, tag="sb2")
                sc, bi = sb2[:, 0:1], sb2[:, 1:2]
                # scale = gamma * rstd ; bias = beta - mean*scale
                nc.vector.tensor_mul(sc, rr, gb_sb[:, p : p + 1])
                nc.vector.tensor_mul(t1, mean, sc)
                nc.vector.tensor_sub(
                    bi, gb_sb[:, N_PAIRS + p : N_PAIRS + p + 1], t1
                )
                return sb2

            def norm_slice(p, y_flat, sb2, h2, nq):
                qf = FREE // nq
                sc, bi = sb2[:, 0:1], sb2[:, 1:2]
                ysl = y_flat[:, h2 * qf : (h2 + 1) * qf]
                nc.scalar.activation(
                    out=ysl, in_=ysl, func=AF.Relu, scale=sc, bias=bi,
                )
                nc.gpsimd.dma_start(
                    out=out[p][:, h2 * qf : (h2 + 1) * qf], in_=ysl
                )

            # ---------- main PE-pair loop (software-pipelined stats) ----------
            gb_loaded = False
            pend = [None]  # (p, y, sb2) awaiting normalize
            for p in range(N_PE_PAIRS):
                ci = p % CH_PER_CORE

                if p < CH_PER_CORE:
                    nc.sync.dma_start(out=band_sb[:, ci], in_=bands[:, ci])
                xt = xpool.tile([H, DP, WP], f16, tag="xt")
                nc.sync.dma_start(out=xt[:, 0:34], in_=xs[p][:, 0:34])
                nc.sync.dma_start(out=xt[:, 34:66], in_=xs[p][:, 34:66])
                if not gb_loaded:
                    nc.sync.dma_start(out=gb_sb[:], in_=gb[:])
                    nc.sync.dma_start(out=wd_sb[:], in_=wd[:])
                    gb_loaded = True
                if p == 0:
                    plan_dve_pair(0)
                if p == 5:
                    plan_dve_pair(1)

                y = ypool.tile([H, FREE], f16, tag="y")
                ss = stats.tile([128, 2, N_GROUPS], f32, tag="ss")
                st2 = stats.tile([128, 2], f32, tag="st2")

                # ---- depthwise conv: banded matmuls, PSUM-accumulated
                c0 = 0
                for g, gsz_c in enumerate(GROUP_SIZES):
                    gsz = gsz_c * CD * W
                    ps = psum_mm.tile([128, 4 * CD * W], f32, tag="mm",
                                      name=f"mm_{p}_{g}")
                    for j in range(9):
                        kd, kw = TAPS[j]
                        for c in range(gsz_c):
                            d0 = (c0 + c) * CD
                            nc.tensor.matmul(
                                ps[:, c * CD * W : (c + 1) * CD * W],
                                band_sb[:, ci, j],
                                xt[:, d0 + kd : d0 + kd + CD, kw : kw + W],
                                start=(j == 0), stop=(j == 8),
                                skip_group_check=True,
                            )
                    # ---- evict group: PSUM -> y (fp16), sum(y) via accum_out
                    ysl = y[:, c0 * CD * W : c0 * CD * W + gsz]
                    nc.scalar.activation(
                        out=ysl, in_=ps[:, 0:gsz], func=AF.Copy,
                        accum_out=ss[:, 0, g : g + 1],
                    )
                    # ---- sum(y^2) for the group: ScalarE Square, discard out
                    nc.scalar.activation(
                        out=scrap[:, 0:gsz], in_=ysl, func=AF.Square,
                        accum_out=ss[:, 1, g : g + 1],
                    )
                    c0 += gsz_c
                    # previous pair's normalize, spread across this section
                    if pend[0] is not None and g in (1, 2):
                        pp_, py_, psb_ = pend[0]
                        norm_slice(pp_, py_, psb_, g - 1, 2)
                        if g == 2:
                            pend[0] = None

                nc.vector.tensor_reduce(
                    out=st2[:], in_=ss[:], axis=mybir.AxisListType.X, op=OP.add,
                )
                red = stats_phase1(st2)
                emit_dve(6)
                sb2 = stats_phase2(p, red)
                emit_dve(6 if p > 0 else 2)
                if p < N_PE_PAIRS - 1:
                    pend[0] = (p, y, sb2)
                else:
                    # last pair: normalize immediately, finer slices
                    for h2 in range(4):
                        norm_slice(p, y[:], sb2, h2, 4)
            emit_dve(len(dve_ops))

    nc.compile()
    return nc


_NC_CACHE = None


def _get_program():
    global _NC_CACHE
    if _NC_CACHE is None:
        _NC_CACHE = build_program()
    return _NC_CACHE


def _band_mats(w, c):
    """f32 band matrices [9, H, H] for channel c, tap order TAPS."""
    eye0 = np.eye(H, dtype=np.float32)
    eyep = np.eye(H, k=1, dtype=np.float32)  # B[h-1, h]: kh=0 tap
    eyem = np.eye(H, k=-1, dtype=np.float32)  # B[h+1, h]: kh=2 tap
    mats = np.empty((9, H, H), np.float32)
    for t, (kd, kw) in enumerate(TAPS):
        wk = w[c, 0, kd, :, kw]
        mats[t] = wk[0] * eyep + wk[1] * eye0 + wk[2] * eyem
    return mats


def make_core_inputs(x, w, gamma, beta, core):
    cs = slice(CH_PER_CORE * core, CH_PER_CORE * (core + 1))
    # (b, ci, d, h, w) -> (b, ci, h, d, w) -> (pair, h, d, w), pair = b*8+ci
    xt = (
        np.ascontiguousarray(x[:, cs].transpose(0, 1, 3, 2, 4))
        .reshape(N_PAIRS, H, D, W)
    )
    xc = np.zeros((N_PE_PAIRS, H, DP, WP), np.float16)
    xc[:, :, 1 : D + 1, 1 : W + 1] = xt[:N_PE_PAIRS].astype(np.float16)
    bpk = np.zeros((H, CH_PER_CORE, 9, H), np.float32)
    for ci in range(CH_PER_CORE):
        bpk[:, ci] = _band_mats(w, CH_PER_CORE * core + ci).transpose(1, 0, 2)
    band_arr = bpk.astype(np.float16)
    gbv = np.broadcast_to(
        np.concatenate([np.tile(gamma[cs], B), np.tile(beta[cs], B)])
        .astype(np.float32)
        .reshape(1, 2 * N_PAIRS),
        (128, 2 * N_PAIRS),
    ).copy()

    # DVE pairs: 3 h-shifted padded copies, two d-halves each
    xdv = np.zeros((N_DVE_PAIRS, 2, H, 3, DHP, WP), np.float16)
    wcols = np.zeros((128, N_DVE_PAIRS * 27), np.float32)
    for dp in range(N_DVE_PAIRS):
        pair = N_PE_PAIRS + dp
        b_i, ci = pair // CH_PER_CORE, pair % CH_PER_CORE
        ch = CH_PER_CORE * core + ci
        xp = np.zeros((H + 2, DP, WP), np.float16)
        xp[1 : H + 1, 1 : D + 1, 1 : W + 1] = (
            x[b_i, ch].transpose(1, 0, 2).astype(np.float16)
        )
        for k in range(3):
            shifted = xp[k : k + H]  # h+ (k-1)
            for h in range(2):
                xdv[dp, h, :, k] = shifted[:, h * DH : h * DH + DHP]
        wcols[:, dp * 27 : (dp + 1) * 27] = np.broadcast_to(
            w[ch, 0].reshape(1, 27).astype(np.float32), (128, 27)
        )
    return {"xs": xc, "bands": band_arr, "gb": gbv, "xd": xdv, "wd": wcols}


def kernel(x, w, gamma, beta):
    from concourse.bass_utils import run_bass_kernel_spmd

    x = np.asarray(x, dtype=np.float32)
    w = np.asarray(w, dtype=np.float32)
    gamma = np.asarray(gamma, dtype=np.float32)
    beta = np.asarray(beta, dtype=np.float32)

    nc = _get_program()
    in_maps = [make_core_inputs(x, w, gamma, beta, k) for k in range(N_CORES)]
    res = run_bass_kernel_spmd(nc, in_maps, core_ids=list(range(N_CORES)))

    out = np.empty((B, C, D, H, W), np.float32)
    for k in range(N_CORES):
        cs = slice(CH_PER_CORE * k, CH_PER_CORE * (k + 1))
        yc = (
            res.results[k]["out"]
            .astype(np.float32)
            .reshape(B, CH_PER_CORE, H, D, W)
        )
        out[:, cs] = yc.transpose(0, 1, 3, 2, 4)
    return out


# revision 10
# speedup vs baseline: 1.0626x; 1.0213x over previous
"""Trainium2 Bass kernel for DWConvBlock3D:
depthwise 3x3x3 conv (pad 1) + InstanceNorm3d + ReLU on x:(2,64,64,128,128) f32.

Strategy (8 NeuronCores, channel sharding => zero communication):
  - Each core owns 8 channels x 2 batches = 16 (b,c) "pairs".
  - Layout per pair: H=128 on SBUF partitions, (D,W) on the free dim, with
    host-side zero padding in both D (66) and W (130) so every matmul is
    uniform (no edge clipping).
  - 14 pairs run on TensorE as banded matmuls: a 128x128 banded matrix
    (3 diagonals = the kh taps) multiplies a (d,w)-shifted view of the x
    tile; shifts cover (kd,kw) -> 9 matmuls per 512-col chunk.
  - 2 pairs run on the otherwise-idle DVE: the host ships 3 h-shifted
    copies (split in d-halves to bound SBUF) so all 27 taps are
    free-dim-shifted ops.  Aligned taps (kw 0/2) use tensor_scalar (4x)
    + tensor_tensor add (2x) = 6.5us/tap-volume; kw=1 taps (odd offset,
    no 2x uop for STT) use scalar_tensor_tensor at 1x.  Ops are
    interleaved a few per PE-pair section so the DVE queue never starves.
  - PSUM: mm pool 2 bufs x 4 banks (groups of 4 chunks; eviction =
    ScalarE activation-copy fp32->fp16 with accum_out -> sum(y)).
  - sum(y^2): ScalarE Square-activation per group (accum_out), scrap
    output discarded -> keeps DVE free for conv taps.
  - cross-partition reduction of (sum, sumsq): GpSimd partition_all_reduce
    issued as "phase1" right after the per-pair reduce, with conv taps
    emitted between it and the dependent "phase2" scale/bias math so the
    DVE never idles on the GpSimd latency; the normalize of pair p is
    deferred into pair p+1's section (emitted after its early evictions)
    so a waiting normalize never head-blocks the ScalarE FIFO.
  - final normalize+ReLU: ScalarE activation with per-partition
    scale/bias; output DMA'd as fp16.

Measured notes (HW probes): matmul streams 1 col/cycle @2.4GHz for all
dtypes; fp8 DoubleRow/DoublePixel give no column-rate win (DoublePixel is
silently ignored); DVE STT has only a 1x uop (8.7us per full volume);
TT=2x, TS=4x; ScalarE is strictly 1x (no Accel modes).
"""

import sys

if "/opt/trn_rl_repo" not in sys.path:
    sys.path.insert(0, "/opt/trn_rl_repo")

import numpy as np

B, C, D, H, W = 2, 64, 64, 128, 128
N_CORES = 8
CH_PER_CORE = C // N_CORES  # 8
N_PAIRS = B * CH_PER_CORE  # 16
DP = D + 2  # host-padded D
WP = W + 2  # host-padded W
FREE = D * W  # 8192 output cols per partition per pair
NV = D * H * W  # normalization element count per (b,c)
EPS = 1e-5
CD = 4  # d-slices per chunk (4*128 = 512 fp32 = 1 PSUM bank)
GROUP_SIZES = [4, 4, 4, 4]  # chunks per PSUM group (4 banks each)
N_CHUNKS = D // CD  # 16
N_GROUPS = len(GROUP_SIZES)  # 6
TAPS = [(kd, kw) for kd in range(3) for kw in range(3)]
N_DVE_PAIRS = 2
N_PE_PAIRS = N_PAIRS - N_DVE_PAIRS  # 14
DH = D // 2  # d-half for DVE pairs
DHP = DH + 2  # padded rows per half


def build_program():
    import concourse.bacc as bacc
    import concourse.mybir as mybir
    from concourse.tile import TileContext

    from concourse import bass_isa

    f32 = mybir.dt.float32
    f16 = mybir.dt.float16
    AF = mybir.ActivationFunctionType
    OP = mybir.AluOpType
    nc = bacc.Bacc("TRN2", target_bir_lowering=False, debug=False, num_devices=N_CORES)

    xs = nc.dram_tensor("xs", [N_PE_PAIRS, H, DP, WP], f16, kind="ExternalInput")
    bands = nc.dram_tensor("bands", [H, CH_PER_CORE, 9, H], f16, kind="ExternalInput")
    gb = nc.dram_tensor("gb", [128, 2 * N_PAIRS], f32, kind="ExternalInput")
    # DVE pairs: 3 h-shifted copies, split in two d-halves (34 padded rows)
    xd = nc.dram_tensor("xd", [N_DVE_PAIRS, 2, H, 3, DHP, WP], f16,
                        kind="ExternalInput")
    wd = nc.dram_tensor("wd", [128, N_DVE_PAIRS * 27], f32, kind="ExternalInput")
    out = nc.dram_tensor("out", [N_PAIRS, H, FREE], f16, kind="ExternalOutput")

    with TileContext(nc) as tc:
        with (
            tc.tile_pool(name="singles", bufs=1) as singles,
            tc.tile_pool(name="xp", bufs=2) as xpool,
            tc.tile_pool(name="yp", bufs=3) as ypool,
            tc.tile_pool(name="xd", bufs=2) as xdpool,
            tc.tile_pool(name="yd", bufs=2) as ydpool,
            tc.tile_pool(name="st", bufs=4) as stats,
            tc.tile_pool(name="psmm", bufs=2, space="PSUM") as psum_mm,
        ):
            band_sb = singles.tile([H, CH_PER_CORE, 9, H], f16)
            gb_sb = singles.tile([128, 2 * N_PAIRS], f32)
            wd_sb = singles.tile([128, N_DVE_PAIRS * 27], f32)
            scrap = singles.tile([128, 4 * CD * W], f16)  # Square-act output
            sq4 = singles.tile([128, DH * W], f16)  # Square output, DVE halves
            tmp = singles.tile([H, DH * W], f16)  # DVE TS scratch (half vol)

            # ---------- DVE-pair machinery ----------
            dve_ops = []  # list of closures, emitted a few per PE section

            def plan_dve_pair(dp):
                """Queue all DVE ops for DVE-pair index dp (pair N_PE_PAIRS+dp)."""
                p = N_PE_PAIRS + dp
                yd_t = ydpool.tile([H, FREE], f16, tag="yd", name=f"yd{dp}")
                sd = stats.tile([128, 2, 2], f32, tag=f"sd{dp}")
                xh_t = [None, None]

                def load_half(h):
                    # fresh-buffer loads issue with ~no WAR wait; pair 15's
                    # loads are planned late (p==7) so their buffer-reuse
                    # wait is short and xt loads behind them aren't stalled
                    xh_t[h] = xdpool.tile([H, 3, DHP, WP], f16, tag="xdh",
                                          name=f"xd{dp}_{h}")
                    nc.sync.dma_start(out=xh_t[h][:], in_=xd[dp, h])

                def do_tap(h, t, first, last):
                    kd, kh, kw = t // 9, (t // 3) % 3, t % 3
                    view = xh_t[h][:, kh, kd : kd + DH, kw : kw + W]
                    ycur = yd_t[:, h * DH * W : (h + 1) * DH * W]
                    wcol = wd_sb[:, dp * 27 + t : dp * 27 + t + 1]
                    if first:
                        nc.vector.tensor_scalar_mul(ycur, view, wcol)
                    elif kw == 1:
                        nc.vector.scalar_tensor_tensor(
                            out=ycur, in0=view, scalar=wcol, in1=ycur,
                            op0=OP.mult, op1=OP.add,
                            accum_out=sd[:, 0, h : h + 1] if last else None,
                        )
                    else:
                        nc.vector.tensor_scalar_mul(tmp[:], view, wcol)
                        nc.vector.tensor_tensor(
                            out=ycur, in0=ycur, in1=tmp[:], op=OP.add
                        )

                def do_sumsq(h):
                    ycur = yd_t[:, h * DH * W : (h + 1) * DH * W]
                    nc.scalar.activation(
                        out=sq4[:], in_=ycur, func=AF.Square,
                        accum_out=sd[:, 1, h : h + 1],
                    )

                # aligned taps (kw 0/2) first, kw=1 (STT, in-place) last so the
                # final op carries accum_out for sum(y)
                order = [t for t in range(27) if t % 3 != 1] + \
                        [t for t in range(27) if t % 3 == 1]
                # both half-loads up front: transfers overlap h0's taps
                dve_ops.append(lambda: load_half(0))
                dve_ops.append(lambda: load_half(1))
                for h in range(2):
                    for i, t in enumerate(order):
                        dve_ops.append(
                            lambda h=h, t=t, fi=(i == 0), la=(i == 26):
                            do_tap(h, t, fi, la)
                        )
                    dve_ops.append(lambda h=h: do_sumsq(h))

                def finish():
                    st2 = stats.tile([128, 2], f32, tag="st2d")
                    nc.vector.tensor_reduce(
                        out=st2[:], in_=sd[:], axis=mybir.AxisListType.X,
                        op=OP.add,
                    )
                    red = stats_phase1(st2)
                    sb2 = stats_phase2(p, red)
                    for h2 in range(2):
                        norm_slice(p, yd_t[:], sb2, h2, 2)

                dve_ops.append(finish)

            def emit_dve(n):
                for _ in range(n):
                    if dve_ops:
                        dve_ops.pop(0)()

            # ---------- shared stats tail ----------
            def stats_phase1(st2):
                """cross-partition reduce of (sum, sumsq) on GpSimd (async)."""
                red = stats.tile([128, 2], f32, tag="red")
                nc.gpsimd.partition_all_reduce(
                    red[:], st2[:], 128, bass_isa.ReduceOp.add
                )
                return red

            def stats_phase2(p, red):
                """red [128,2] broadcast (N*mean, N*ex2) -> sc/bi tiles."""
                sm = stats.tile([128, 10], f32, tag="sm")
                mean, ex2 = sm[:, 0:1], sm[:, 1:2]
                msq, vpe = sm[:, 2:3], sm[:, 3:4]
                std, r0 = sm[:, 4:5], sm[:, 5:6]
                t1, t2 = sm[:, 6:7], sm[:, 7:8]
                t4, rr = sm[:, 8:9], sm[:, 9:10]
                nc.vector.tensor_scalar_mul(mean, red[:, 0:1], 1.0 / NV)
                nc.vector.tensor_scalar_mul(ex2, red[:, 1:2], 1.0 / NV)
                nc.vector.tensor_mul(msq, mean, mean)
                nc.vector.tensor_sub(vpe, ex2, msq)
                nc.vector.tensor_scalar_add(vpe, vpe, EPS)
                nc.scalar.activation(std, vpe, mybir.ActivationFunctionType.Sqrt)
                nc.vector.reciprocal(r0, std)
                # one Newton step: r = r0*(1.5 - 0.5*vpe*r0^2)
                nc.vector.tensor_mul(t1, r0, r0)
                nc.vector.tensor_mul(t2, t1, vpe)
                nc.vector.tensor_scalar(
                    t4, t2, -0.5, 1.5, op0=OP.mult, op1=OP.add,
                )
                nc.vector.tensor_mul(rr, r0, t4)

                sb2 = stats.tile([128, 2], f32, tag="sb2")
                sc, bi = sb2[:, 0:1], sb2[:, 1:2]
                # scale = gamma * rstd ; bias = beta - mean*scale
                nc.vector.tensor_mul(sc, rr, gb_sb[:, p : p + 1])
                nc.vector.tensor_mul(t1, mean, sc)
                nc.vector.tensor_sub(
                    bi, gb_sb[:, N_PAIRS + p : N_PAIRS + p + 1], t1
                )
                return sb2

            def norm_slice(p, y_flat, sb2, h2, nq):
                qf = FREE // nq
                sc, bi = sb2[:, 0:1], sb2[:, 1:2]
                ysl = y_flat[:, h2 * qf : (h2 + 1) * qf]
                nc.scalar.activation(
                    out=ysl, in_=ysl, func=AF.Relu, scale=sc, bias=bi,
                )
                nc.gpsimd.dma_start(
                    out=out[p][:, h2 * qf : (h2 + 1) * qf], in_=ysl
                )

            # ---------- main PE-pair loop (software-pipelined stats) ----------
            gb_loaded = False
            pend = [None]  # (p, y, sb2) awaiting normalize
            for p in range(N_PE_PAIRS):
                ci = p % CH_PER_CORE

                if p < CH_PER_CORE:
                    nc.sync.dma_start(out=band_sb[:, ci], in_=bands[:, ci])
                xt = xpool.tile([H, DP, WP], f16, tag="xt")
                nc.sync.dma_start(out=xt[:, 0:34], in_=xs[p][:, 0:34])
                nc.sync.dma_start(out=xt[:, 34:66], in_=xs[p][:, 34:66])
                if not gb_loaded:
                    nc.sync.dma_start(out=gb_sb[:], in_=gb[:])
                    nc.sync.dma_start(out=wd_sb[:], in_=wd[:])
                    gb_loaded = True
                if p == 0:
                    plan_dve_pair(0)
                if p == 7:
                    plan_dve_pair(1)

                y = ypool.tile([H, FREE], f16, tag="y")
                ss = stats.tile([128, 2, N_GROUPS], f32, tag="ss")
                st2 = stats.tile([128, 2], f32, tag="st2")

                # ---- depthwise conv: banded matmuls, PSUM-accumulated
                c0 = 0
                for g, gsz_c in enumerate(GROUP_SIZES):
                    gsz = gsz_c * CD * W
                    ps = psum_mm.tile([128, 4 * CD * W], f32, tag="mm",
                                      name=f"mm_{p}_{g}")
                    for j in range(9):
                        kd, kw = TAPS[j]
                        for c in range(gsz_c):
                            d0 = (c0 + c) * CD
                            nc.tensor.matmul(
                                ps[:, c * CD * W : (c + 1) * CD * W],
                                band_sb[:, ci, j],
                                xt[:, d0 + kd : d0 + kd + CD, kw : kw + W],
                                start=(j == 0), stop=(j == 8),
                                skip_group_check=True,
                            )
                    # ---- evict group: PSUM -> y (fp16), sum(y) via accum_out
                    ysl = y[:, c0 * CD * W : c0 * CD * W + gsz]
                    nc.scalar.activation(
                        out=ysl, in_=ps[:, 0:gsz], func=AF.Copy,
                        accum_out=ss[:, 0, g : g + 1],
                    )
                    # ---- sum(y^2) for the group: ScalarE Square, discard out
                    nc.scalar.activation(
                        out=scrap[:, 0:gsz], in_=ysl, func=AF.Square,
                        accum_out=ss[:, 1, g : g + 1],
                    )
                    c0 += gsz_c
                    # previous pair's normalize, spread across this section
                    if pend[0] is not None and g in (1, 2):
                        pp_, py_, psb_ = pend[0]
                        norm_slice(pp_, py_, psb_, g - 1, 2)
                        if g == 2:
                            pend[0] = None

                nc.vector.tensor_reduce(
                    out=st2[:], in_=ss[:], axis=mybir.AxisListType.X, op=OP.add,
                )
                red = stats_phase1(st2)
                emit_dve(6)
                sb2 = stats_phase2(p, red)
                emit_dve(6 if p > 0 else 2)
                if p < N_PE_PAIRS - 1:
                    pend[0] = (p, y, sb2)
                else:
                    # last pair: normalize immediately, finer slices
                    for h2 in range(4):
                        norm_slice(p, y[:], sb2, h2, 4)
            emit_dve(len(dve_ops))

    nc.compile()
    return nc


_NC_CACHE = None


def _get_program():
    global _NC_CACHE
    if _NC_CACHE is None:
        _NC_CACHE = build_program()
    return _NC_CACHE


def _band_mats(w, c):
    """f32 band matrices [9, H, H] for channel c, tap order TAPS."""
    eye0 = np.eye(H, dtype=np.float32)
    eyep = np.eye(H, k=1, dtype=np.float32)  # B[h-1, h]: kh=0 tap
    eyem = np.eye(H, k=-1, dtype=np.float32)  # B[h+1, h]: kh=2 tap
    mats = np.empty((9, H, H), np.float32)
    for t, (kd, kw) in enumerate(TAPS):
        wk = w[c, 0, kd, :, kw]
        mats[t] = wk[0] * eyep + wk[1] * eye0 + wk[2] * eyem
    return mats


def make_core_inputs(x, w, gamma, beta, core):
    cs = slice(CH_PER_CORE * core, CH_PER_CORE * (core + 1))
    # (b, ci, d, h, w) -> (b, ci, h, d, w) -> (pair, h, d, w), pair = b*8+ci
    xt = (
        np.ascontiguousarray(x[:, cs].transpose(0, 1, 3, 2, 4))
        .reshape(N_PAIRS, H, D, W)
    )
    xc = np.zeros((N_PE_PAIRS, H, DP, WP), np.float16)
    xc[:, :, 1 : D + 1, 1 : W + 1] = xt[:N_PE_PAIRS].astype(np.float16)
    bpk = np.zeros((H, CH_PER_CORE, 9, H), np.float32)
    for ci in range(CH_PER_CORE):
        bpk[:, ci] = _band_mats(w, CH_PER_CORE * core + ci).transpose(1, 0, 2)
    band_arr = bpk.astype(np.float16)
    gbv = np.broadcast_to(
        np.concatenate([np.tile(gamma[cs], B), np.tile(beta[cs], B)])
        .astype(np.float32)
        .reshape(1, 2 * N_PAIRS),
        (128, 2 * N_PAIRS),
    ).copy()

    # DVE pairs: 3 h-shifted padded copies, two d-halves each
    xdv = np.zeros((N_DVE_PAIRS, 2, H, 3, DHP, WP), np.float16)
    wcols = np.zeros((128, N_DVE_PAIRS * 27), np.float32)
    for dp in range(N_DVE_PAIRS):
        pair = N_PE_PAIRS + dp
        b_i, ci = pair // CH_PER_CORE, pair % CH_PER_CORE
        ch = CH_PER_CORE * core + ci
        xp = np.zeros((H + 2, DP, WP), np.float16)
        xp[1 : H + 1, 1 : D + 1, 1 : W + 1] = (
            x[b_i, ch].transpose(1, 0, 2).astype(np.float16)
        )
        for k in range(3):
            shifted = xp[k : k + H]  # h+ (k-1)
            for h in range(2):
                xdv[dp, h, :, k] = shifted[:, h * DH : h * DH + DHP]
        wcols[:, dp * 27 : (dp + 1) * 27] = np.broadcast_to(
            w[ch, 0].reshape(1, 27).astype(np.float32), (128, 27)
        )
    return {"xs": xc, "bands": band_arr, "gb": gbv, "xd": xdv, "wd": wcols}


def kernel(x, w, gamma, beta):
    from concourse.bass_utils import run_bass_kernel_spmd

    x = np.asarray(x, dtype=np.float32)
    w = np.asarray(w, dtype=np.float32)
    gamma = np.asarray(gamma, dtype=np.float32)
    beta = np.asarray(beta, dtype=np.float32)

    nc = _get_program()
    in_maps = [make_core_inputs(x, w, gamma, beta, k) for k in range(N_CORES)]
    res = run_bass_kernel_spmd(nc, in_maps, core_ids=list(range(N_CORES)))

    out = np.empty((B, C, D, H, W), np.float32)
    for k in range(N_CORES):
        cs = slice(CH_PER_CORE * k, CH_PER_CORE * (k + 1))
        yc = (
            res.results[k]["out"]
            .astype(np.float32)
            .reshape(B, CH_PER_CORE, H, D, W)
        )
        out[:, cs] = yc.transpose(0, 1, 3, 2, 4)
    return out
